# revision 9
# baseline (speedup 1.0000x reference)
"""Windowed sparse attention kernel for TRN2 (8 NeuronCores).

Problem: b=1, h=16, n=16384, d=32, window w=128, nw=128 windows.
Each window of 128 queries attends to [4 memory slots | prev window | cur window]
with additive bias, tanh softcap (50), softmax.

Sharding: sequence-parallel over windows. Core c handles windows
[c*16, (c+1)*16) for all 16 heads, with a one-window k/v halo.

Device dataflow (keys on partitions, slot-major, bf16 matmuls):
  mm1: per k/v slot s (17 per core), one N=256 bf16 matmul computes
  simT[key_s, (q_{s-1} | q_s)] into PSUM. Bias is added either by DVE
  (tensor_tensor add during PSUM->SBUF evacuation, most pairs) or by the
  PE itself (identity-weight matmul accumulating the bias columns into
  the same PSUM region, PE_PAIRS) so ACT's tanh pass doubles as the
  evacuation for those pairs. ACT runs ONE pass (tanh -> fp16); the exp
  is computed on DVE with a Schraudolph bit-trick: int16(t*a + b) is
  exactly the bf16 bit pattern of ~exp(50 t), one tensor_scalar at 4x
  rate. mm2 consumes the int16 tile bitcast to bf16:
  lhsT = p-slice (keys x queries), rhs = v~ (keys x 33) -> out (128 q, 33)
  per task, where v~'s ones column makes col 32 the softmax denominator Z.
  ACT evacuates mm2's PSUM to bf16 outW; host combines the 4-slot memory
  attention and normalizes in float64.
"""

import numpy as np
import ml_dtypes

B, H, N, D = 1, 16, 16384, 32
W = 128                 # window size
NW = N // W             # 128 windows
NCORES = 8
WPC = NW // NCORES      # 16 windows (tasks) per core
NSLOT = WPC + 1         # 17 k/v slots (halo)
SOFTCLAMP = 50.0
SCALE = D ** -0.5
MASK_PEN = -30000.0
SIMW = NSLOT * 256      # 4352 wide-tile cols (slot-major, 256 per slot)
CHUNK = 1536            # PSUM evacuation chunk (3 banks, 6 slots)

# Schraudolph exp: bf16 bits of exp(50*t) ~= round(t * 50*128/ln2 + 16256 + C)
A_CONST = float(50.0 * 128.0 / np.log(2.0))
B_CONST = 16256.0 - 8.0

# pairs whose bias-add runs on the PE (identity matmul into PSUM) instead
# of DVE; ACT's tanh then reads PSUM directly (fused evacuation)
PE_PAIRS = (3, 6)

BF16 = ml_dtypes.bfloat16

_COMPILED = None


def _build_bass():
    import concourse.bacc as bacc
    import concourse.tile as tile
    from concourse import mybir
    from contextlib import ExitStack

    f32 = mybir.dt.float32
    f16 = mybir.dt.float16
    i16 = mybir.dt.int16
    bf16 = mybir.dt.bfloat16
    nc = bacc.Bacc()

    qT = nc.declare_dram_parameter("qT", [4, 128, WPC * W], bf16, isOutput=False)
    kT = nc.declare_dram_parameter("kT", [4, 128, NSLOT * W], bf16, isOutput=False)
    vv = nc.declare_dram_parameter("vv", [H, 128, NSLOT * 33], bf16, isOutput=False)
    bT = nc.declare_dram_parameter("bT", [128, SIMW], bf16, isOutput=False)
    ident = nc.declare_dram_parameter("ident", [128, 128], bf16, isOutput=False)
    o = nc.declare_dram_parameter("o", [H, 128, WPC * 33], bf16, isOutput=True)

    # chunk layout: (col_start, ncols, slots)
    chunks = [(0, 1536, list(range(0, 6))),
              (1536, 1536, list(range(6, 12))),
              (3072, 1280, list(range(12, 17)))]

    with ExitStack() as ctx:
        tc = ctx.enter_context(tile.TileContext(nc))
        singles = ctx.enter_context(tc.tile_pool(name="singles", bufs=1))
        qk_pool = ctx.enter_context(tc.tile_pool(name="qk", bufs=2))
        v_pool = ctx.enter_context(tc.tile_pool(name="v", bufs=4))
        sim_pool = ctx.enter_context(tc.tile_pool(name="sims", bufs=4))
        t_pool = ctx.enter_context(tc.tile_pool(name="ts", bufs=4))
        p_pool = ctx.enter_context(tc.tile_pool(name="ps", bufs=4))
        ow_pool = ctx.enter_context(tc.tile_pool(name="ow", bufs=2))
        sim_ps = ctx.enter_context(tc.tile_pool(name="simps", bufs=2, space="PSUM"))
        out_ps = ctx.enter_context(tc.tile_pool(name="outps", bufs=2, space="PSUM"))

        identT = singles.tile([128, 128], bf16, name="identT")
        nc.sync.dma_start(out=identT[:, :], in_=ident[:, :])

        # per-chunk bias tiles: the first consumer only waits on its own slice
        biasC = [None, None, None]

        def emit_bias(ci):
            cs, csz, _ = chunks[ci]
            bt = singles.tile([128, csz], bf16, name=f"bias{ci}")
            eng = nc.gpsimd if ci == 0 else nc.sync
            eng.dma_start(out=bt[:, :], in_=bT[:, cs:cs + csz])
            biasC[ci] = bt

        # K/Q tiles per PSUM chunk: chunk ci's matmuls gate only on slice ci.
        KSL = [(0, 6 * W), (6 * W, 12 * W), (12 * W, NSLOT * W)]
        QSL = [(0, 6 * W), (5 * W, 12 * W), (11 * W, WPC * W)]
        # used span within chunk ci (excludes filler cols at both wide-tile ends)
        def span(ci):
            cs, csz, _ = chunks[ci]
            lo = 128 if ci == 0 else 0
            hi = csz - 128 if ci == 2 else csz
            return lo, hi


        # one-pair software pipeline: each pair's {schraudolph-exp, mm2,
        # output copy} are emitted in the NEXT pair's section so the DVE's
        # strict-FIFO queue never stalls on ACT's tanh (HOL blocking), and
        # mm2 never blocks the next pair's mm1 in the PE queue.
        def emit_schr(prev):
            pair_p, tS_p, pI_p = prev["pair"], prev["tS"], prev["pI"]
            for h, _ in pair_p:
                # exp via Schraudolph: int16(t*a+b) == bf16 bits of exp(50t)
                nc.vector.tensor_scalar(
                    pI_p[h][:, 128:SIMW - 128],
                    tS_p[h][:, 128:SIMW - 128],
                    A_CONST, B_CONST,
                    mybir.AluOpType.mult, mybir.AluOpType.add)

        def emit_mm2(prev):
            # mm2: out (128 q, 33) per task, 8 tasks per PSUM bank
            for u, (h, _) in enumerate(prev["pair"]):
                Vh = prev["Vhs"][h]
                pB = prev["pI"][h]
                outW = ow_pool.tile([128, WPC * 33], bf16, tag="outW",
                                    name=f"outW{h}")
                for tb in range(2):
                    otP = out_ps.tile([128, 8 * 33], f32, tag="otP",
                                      name=f"otP{h}_{tb}")
                    for uu in range(8):
                        t = 8 * tb + uu
                        # prev: slot t keys, q_t = second half of slot t block
                        nc.tensor.matmul(
                            otP[:, uu * 33:(uu + 1) * 33],
                            lhsT=pB[:, t * 256 + 128:t * 256 + 256].bitcast(bf16),
                            rhs=Vh[:, t * 33:(t + 1) * 33],
                            start=True, stop=False)
                        # cur: slot t+1 keys, q_t = first half of slot t+1 block
                        nc.tensor.matmul(
                            otP[:, uu * 33:(uu + 1) * 33],
                            lhsT=pB[:, (t + 1) * 256:(t + 1) * 256 + 128].bitcast(bf16),
                            rhs=Vh[:, (t + 1) * 33:(t + 2) * 33],
                            start=False, stop=True)
                    nc.scalar.activation(
                        outW[:, tb * 264:(tb + 1) * 264], otP[:, :],
                        mybir.ActivationFunctionType.Copy)
                    nc.sync.dma_start(out=o[h][:, tb * 264:(tb + 1) * 264],
                                      in_=outW[:, tb * 264:(tb + 1) * 264])

        pending = None

        for g in range(4):
            Ks, Qs = [], []
            for ci in range(3):
                # the idle GpSimd engine's preamble ends earliest — its SWDGE
                # issues the critical first transfers ~2 µs sooner than HWDGE
                dma = nc.gpsimd if (g == 0 and ci == 0) else nc.sync
                ks0, ks1 = KSL[ci]
                Kc = qk_pool.tile([128, ks1 - ks0], bf16, tag=f"k{ci}",
                                  name=f"k{ci}g{g}")
                dma.dma_start(out=Kc[:, :], in_=kT[g][:, ks0:ks1])
                qs0, qs1 = QSL[ci]
                Qc = qk_pool.tile([128, qs1 - qs0], bf16, tag=f"q{ci}",
                                  name=f"q{ci}g{g}")
                dma.dma_start(out=Qc[:, :], in_=qT[g][:, qs0:qs1])
                Ks.append(Kc)
                Qs.append(Qc)
                if g == 0:
                    emit_bias(ci)       # need-order: K_ci, Q_ci, bias_ci

            def k_ap(ci, p0, s):
                lo = s * W - KSL[ci][0]
                return Ks[ci][p0:p0 + 32, lo:lo + W]

            def q_ap(ci, p0, w, nw):    # query windows w .. w+nw-1
                lo = w * W - QSL[ci][0]
                return Qs[ci][p0:p0 + 32, lo:lo + nw * W]

            for j in range(2):      # head pairs within group, strips (64j, 64j+32)
                pj = 2 * g + j      # pair index 0..7
                pair = [(4 * g + 2 * j + u, 32 * (2 * j + u)) for u in range(2)]
                on_pe = pj in PE_PAIRS
                # per-head fp32 sim in SBUF (DVE-evacuated pairs only)
                simS = {} if on_pe else {
                    h: sim_pool.tile([128, SIMW], f32, tag="simS",
                                     name=f"simS{h}")
                    for h, _ in pair}
                tS = {h: t_pool.tile([128, SIMW], f16, tag="tS", name=f"tS{h}")
                      for h, _ in pair}

                def emit_mm1(hs):
                    # mm1 (+ bias path) chunk by chunk
                    for ci, (cs, csz, slots) in enumerate(chunks):
                        lo2, hi2 = span(ci)
                        simPs = {h: sim_ps.tile([128, CHUNK], f32, tag="simP",
                                                name=f"simP{h}c{ci}")
                                 for h, _ in hs}
                        if on_pe:
                            # bias lands in PSUM FIRST (identity matmul,
                            # start=True pends each 512-col bank piece), then
                            # the slot matmuls accumulate onto it. start=True
                            # on a later matmul would re-mark the whole zero
                            # region pending and turn sibling accumulation
                            # into overwrite, hence this order.
                            for h, _ in hs:
                                bnds = sorted({lo2, hi2} |
                                              {b for b in (512, 1024)
                                               if lo2 < b < hi2})
                                for a0, a1 in zip(bnds[:-1], bnds[1:]):
                                    nc.tensor.matmul(simPs[h][:, a0:a1],
                                                     lhsT=identT[:, :],
                                                     rhs=biasC[ci][:, a0:a1],
                                                     start=True, stop=False,
                                                     skip_group_check=True)
                        for s in slots:
                            off = s * 256 - cs
                            if s == 0:
                                # cols [0:128] (task -1) are filler: never
                                # computed, skipped by ACT, unread by mm2
                                rgn, qw_, qn = (128, 256), 0, 1
                            elif s == NSLOT - 1:
                                rgn, qw_, qn = (off, off + 128), s - 1, 1
                            else:
                                rgn, qw_, qn = (off, off + 256), s - 1, 2
                            for h, p0 in hs:
                                nc.tensor.matmul(simPs[h][:, rgn[0]:rgn[1]],
                                                 lhsT=k_ap(ci, p0, s),
                                                 rhs=q_ap(ci, p0, qw_, qn),
                                                 start=not on_pe, stop=True,
                                                 skip_group_check=on_pe,
                                                 tile_position=(p0, 0))
                        for h, _ in hs:
                            if on_pe:
                                nc.scalar.activation(
                                    tS[h][:, cs + lo2:cs + hi2],
                                    simPs[h][:, lo2:hi2],
                                    mybir.ActivationFunctionType.Tanh,
                                    scale=1.0 / SOFTCLAMP)
                            else:
                                nc.vector.tensor_add(
                                    simS[h][:, cs + lo2:cs + hi2],
                                    simPs[h][:, lo2:hi2],
                                    biasC[ci][:, lo2:hi2],
                                )

                if pj == 0:
                    # head-serial at the very start: h0's sim completes (and
                    # ACT starts) ~3 µs earlier
                    emit_mm1([pair[0]])
                    emit_mm1([pair[1]])
                else:
                    emit_mm1(pair)
                # previous pair's exp lands on the DVE queue right
                # after this pair's evacuation adds
                pI = {h: p_pool.tile([128, SIMW], i16, tag="pI", name=f"pI{h}")
                      for h, _ in pair}
                if pending is not None:
                    emit_schr(pending)
                # softcap: one ACT pass (tanh -> fp16), skipping the 128
                # filler cols at each end (DVE pairs; PE pairs ran it above)
                for h, _ in pair:
                    if not on_pe:
                        nc.scalar.activation(
                            tS[h][:, 128:SIMW - 128],
                            simS[h][:, 128:SIMW - 128],
                            mybir.ActivationFunctionType.Tanh,
                            scale=1.0 / SOFTCLAMP)
                # V DMAs issued after the K/Q/bias stream (mm2 needs them
                # only ~15 µs later; keeps them off the critical early DMAs)
                Vhs = {}
                for h, p0 in pair:
                    Vh = v_pool.tile([128, NSLOT * 33], bf16, tag="vh",
                                     name=f"vh{h}")
                    nc.sync.dma_start(out=Vh[:, :], in_=vv[h])
                    Vhs[h] = Vh
                if pending is not None:
                    emit_mm2(pending)
                pending = {"pair": pair, "tS": tS, "pI": pI, "Vhs": Vhs}
        # drain the pipeline: last pair, head-interleaved so DVE/PE overlap
        if pending is not None:
            last = pending
            for u, (h, p0) in enumerate(last["pair"]):
                one = {"pair": [(h, p0)], "tS": last["tS"], "pI": last["pI"],
                       "Vhs": last["Vhs"]}
                emit_schr(one)
                emit_mm2(one)
    nc.compile()
    return nc


def _get_compiled():
    global _COMPILED
    if _COMPILED is None:
        _COMPILED = _build_bass()
    return _COMPILED


def _prep_core(c, qs, ks, vs, ab, mvec):
    """Build per-core input arrays. qs,ks,vs: (H, N, D) (qs pre-scaled)."""
    w0 = c * WPC
    qw = qs.reshape(H, NW, W, D)[:, w0:w0 + WPC]          # (H,16,128,32)
    qTc = np.ascontiguousarray(
        qw.reshape(4, 4, WPC, W, D).transpose(0, 1, 4, 2, 3).reshape(4, 128, WPC * W))

    kw = ks.reshape(H, NW, W, D)
    vw = vs.reshape(H, NW, W, D)
    khalo = np.zeros((H, NSLOT, W, D), np.float32)
    vhalo = np.zeros((H, NSLOT, W, D), np.float32)
    lo = w0 - 1
    src_lo = max(lo, 0)
    dst_lo = src_lo - lo
    khalo[:, dst_lo:] = kw[:, src_lo:w0 + WPC]
    vhalo[:, dst_lo:] = vw[:, src_lo:w0 + WPC]
    kTc = np.ascontiguousarray(
        khalo.reshape(4, 4, NSLOT, W, D).transpose(0, 1, 4, 2, 3).reshape(4, 128, NSLOT * W))
    vvc = np.concatenate([vhalo, np.ones((H, NSLOT, W, 1), np.float32)], axis=3)
    vvc = np.ascontiguousarray(
        vvc.transpose(0, 2, 1, 3).reshape(H, 128, NSLOT * 33))

    # bias, slot-major: slot s block cols = [cur-bias(task s-1) | prev-bias(task s)]
    # both halves use keys of global window w0+s-1; fold key mask (+ structural
    # masking of window -1) as additive penalty.
    bTc = np.zeros((128, NSLOT, 2, W), np.float32)         # (key, slot, half, q)
    for s in range(NSLOT):
        gw = w0 + s - 1
        if s > 0:
            bTc[:, s, 0, :] = ab[gw, :, 128:256].T          # cur role for task s-1
        if s < NSLOT - 1:
            bTc[:, s, 1, :] = ab[gw + 1, :, 0:128].T        # prev role for task s
        if gw < 0:
            pen = np.full((W,), MASK_PEN, np.float32)
        else:
            pen = np.where(mvec[gw * W:(gw + 1) * W], np.float32(0),
                           np.float32(MASK_PEN))
        bTc[:, s, :, :] += pen[:, None, None]
    bTc = np.ascontiguousarray(bTc.reshape(128, SIMW))
    return {"qT": qTc.astype(BF16), "kT": kTc.astype(BF16),
            "vv": vvc.astype(BF16), "bT": bTc.astype(BF16),
            "ident": np.eye(128, dtype=BF16)}


def _run_device(in_maps, trace=False):
    from concourse.bass_utils import run_bass_kernel_spmd
    nc = _get_compiled()
    res = run_bass_kernel_spmd(nc, in_maps, list(range(NCORES)), trace=trace)
    return res


def kernel(q, k, v, mask, attn_bias, memory_kv, _trace=False, _ret_res=False):
    q = np.asarray(q, np.float32)
    k = np.asarray(k, np.float32)
    v = np.asarray(v, np.float32)
    mask = np.asarray(mask)
    attn_bias = np.asarray(attn_bias, np.float32)
    memory_kv = np.asarray(memory_kv, np.float32)

    qs = q[0] * np.float32(SCALE)       # (H, N, D)
    ks, vs = k[0], v[0]
    ab = attn_bias[0]                   # (NW, W, 2W)
    mvec = mask[0].astype(bool)         # (N,)

    in_maps = [_prep_core(c, qs, ks, vs, ab, mvec) for c in range(NCORES)]
    res = _run_device(in_maps, trace=_trace)
    outs = [np.asarray(r["o"], dtype=np.float32) for r in res.results]

    big = np.stack(outs)                              # (8, H, 128, 528)
    # (core, h, q, task, 33) -> (h, core, task, q, 33) -> (h, n, 33)
    arr = big.reshape(NCORES, H, W, WPC, 33).transpose(1, 0, 3, 2, 4)
    arr = arr.reshape(H, N, 33)
    num = arr[..., :D].astype(np.float64)             # (H, N, D)
    z = arr[..., D].astype(np.float64)                # (H, N)

    # memory-slot attention (4 keys, no bias, mask=True) on host
    mk, mv = memory_kv[0], memory_kv[1]               # (H, 4, D)
    sim_m = np.einsum('hnd,hmd->hnm', qs, mk, dtype=np.float64)
    pm = np.exp(SOFTCLAMP * np.tanh(sim_m / SOFTCLAMP))
    num = num + np.einsum('hnm,hmd->hnd', pm, mv.astype(np.float64))
    z = z + pm.sum(-1)

    out = (num / z[..., None]).astype(np.float32)[None]   # (1, H, N, D)
    if _ret_res:
        return out, res
    return out


# revision 10
# speedup vs baseline: 1.0867x; 1.0867x over previous
"""Windowed sparse attention kernel for TRN2 (8 NeuronCores).

Problem: b=1, h=16, n=16384, d=32, window w=128, nw=128 windows.
Each window of 128 queries attends to [4 memory slots | prev window | cur window]
with additive bias, tanh softcap (50), softmax.

Sharding: sequence-parallel over windows. Core c handles windows
[c*16, (c+1)*16) for all 16 heads, with a one-window k/v halo.

Device dataflow (keys on partitions, slot-major, bf16 matmuls):
  mm1: per k/v slot s (17 per core), one N=256 bf16 matmul computes
  simT[key_s, (q_{s-1} | q_s)] into PSUM. Bias is added either by DVE
  (tensor_tensor add during PSUM->SBUF evacuation, most pairs) or by the
  PE itself (identity-weight matmul accumulating the bias columns into
  the same PSUM region, PE_PAIRS) so ACT's tanh pass doubles as the
  evacuation for those pairs. ACT runs ONE pass (tanh -> fp16); the exp
  is computed on DVE with a Schraudolph bit-trick: int16(t*a + b) is
  exactly the bf16 bit pattern of ~exp(50 t), one tensor_scalar at 4x
  rate. mm2 consumes the int16 tile bitcast to bf16:
  lhsT = p-slice (keys x queries), rhs = v~ (keys x 33) -> out (128 q, 33)
  per task, where v~'s ones column makes col 32 the softmax denominator Z.
  ACT evacuates mm2's PSUM to bf16 outW; host combines the 4-slot memory
  attention and normalizes in float64.
"""

import numpy as np
import ml_dtypes

B, H, N, D = 1, 16, 16384, 32
W = 128                 # window size
NW = N // W             # 128 windows
NCORES = 8
WPC = NW // NCORES      # 16 windows (tasks) per core
NSLOT = WPC + 1         # 17 k/v slots (halo)
SOFTCLAMP = 50.0
SCALE = D ** -0.5
MASK_PEN = -30000.0
SIMW = NSLOT * 256      # 4352 wide-tile cols (slot-major, 256 per slot)
CHUNK = 1536            # PSUM evacuation chunk (3 banks, 6 slots)

# Schraudolph exp: bf16 bits of exp(50*t) ~= round(t * 50*128/ln2 + 16256 + C)
A_CONST = float(50.0 * 128.0 / np.log(2.0))
B_CONST = 16256.0 - 8.0

# pairs whose bias-add runs on the PE (identity matmul into PSUM) instead
# of DVE; ACT's tanh then reads PSUM directly (fused evacuation)
PE_PAIRS = ()

BF16 = ml_dtypes.bfloat16

_COMPILED = None


def _build_bass():
    import concourse.bacc as bacc
    import concourse.tile as tile
    from concourse import mybir
    from contextlib import ExitStack

    f32 = mybir.dt.float32
    f16 = mybir.dt.float16
    i16 = mybir.dt.int16
    bf16 = mybir.dt.bfloat16
    nc = bacc.Bacc()

    qT = nc.declare_dram_parameter("qT", [4, 128, WPC * W], bf16, isOutput=False)
    kT = nc.declare_dram_parameter("kT", [4, 128, NSLOT * W], bf16, isOutput=False)
    vv = nc.declare_dram_parameter("vv", [H, 128, NSLOT * 33], bf16, isOutput=False)
    bT = nc.declare_dram_parameter("bT", [128, SIMW], bf16, isOutput=False)
    ident = nc.declare_dram_parameter("ident", [128, 128], bf16, isOutput=False)
    o = nc.declare_dram_parameter("o", [H, 128, WPC * 33], bf16, isOutput=True)

    # chunk layout: (col_start, ncols, slots)
    chunks = [(0, 1536, list(range(0, 6))),
              (1536, 1536, list(range(6, 12))),
              (3072, 1280, list(range(12, 17)))]

    with ExitStack() as ctx:
        tc = ctx.enter_context(tile.TileContext(nc))
        singles = ctx.enter_context(tc.tile_pool(name="singles", bufs=1))
        qk_pool = ctx.enter_context(tc.tile_pool(name="qk", bufs=2))
        v_pool = ctx.enter_context(tc.tile_pool(name="v", bufs=4))
        sim_pool = ctx.enter_context(tc.tile_pool(name="sims", bufs=4))
        t_pool = ctx.enter_context(tc.tile_pool(name="ts", bufs=4))
        p_pool = ctx.enter_context(tc.tile_pool(name="ps", bufs=4))
        ow_pool = ctx.enter_context(tc.tile_pool(name="ow", bufs=2))
        sim_ps = ctx.enter_context(tc.tile_pool(name="simps", bufs=2, space="PSUM"))
        out_ps = ctx.enter_context(tc.tile_pool(name="outps", bufs=2, space="PSUM"))

        identT = singles.tile([128, 128], bf16, name="identT")
        nc.sync.dma_start(out=identT[:, :], in_=ident[:, :])

        # per-chunk bias tiles: the first consumer only waits on its own slice
        biasC = [None, None, None]

        def emit_bias(ci):
            cs, csz, _ = chunks[ci]
            bt = singles.tile([128, csz], bf16, name=f"bias{ci}")
            eng = nc.gpsimd if ci == 0 else nc.sync
            eng.dma_start(out=bt[:, :], in_=bT[:, cs:cs + csz])
            biasC[ci] = bt

        # K/Q tiles per PSUM chunk: chunk ci's matmuls gate only on slice ci.
        KSL = [(0, 6 * W), (6 * W, 12 * W), (12 * W, NSLOT * W)]
        QSL = [(0, 6 * W), (5 * W, 12 * W), (11 * W, WPC * W)]
        # used span within chunk ci (excludes filler cols at both wide-tile ends)
        def span(ci):
            cs, csz, _ = chunks[ci]
            lo = 128 if ci == 0 else 0
            hi = csz - 128 if ci == 2 else csz
            return lo, hi


        # one-pair software pipeline: each pair's {schraudolph-exp, mm2,
        # output copy} are emitted in the NEXT pair's section so the DVE's
        # strict-FIFO queue never stalls on ACT's tanh (HOL blocking), and
        # mm2 never blocks the next pair's mm1 in the PE queue.
        def emit_schr(prev):
            pair_p, tS_p, pI_p = prev["pair"], prev["tS"], prev["pI"]
            for h, _ in pair_p:
                # exp via Schraudolph: int16(t*a+b) == bf16 bits of exp(50t)
                nc.vector.tensor_scalar(
                    pI_p[h][:, 128:SIMW - 128],
                    tS_p[h][:, 128:SIMW - 128],
                    A_CONST, B_CONST,
                    mybir.AluOpType.mult, mybir.AluOpType.add)

        def emit_mm2(prev, tbs=(0, 1)):
            # mm2: out (128 q, 33) per task, 8 tasks per PSUM bank
            for u, (h, _) in enumerate(prev["pair"]):
                Vh = prev["Vhs"][h]
                pB = prev["pI"][h]
                outW = ow_pool.tile([128, WPC * 33], bf16, tag="outW",
                                    name=f"outW{h}t{tbs[0]}")
                for tb in tbs:
                    otP = out_ps.tile([128, 8 * 33], f32, tag="otP",
                                      name=f"otP{h}_{tb}")
                    for uu in range(8):
                        t = 8 * tb + uu
                        # prev: slot t keys, q_t = second half of slot t block
                        nc.tensor.matmul(
                            otP[:, uu * 33:(uu + 1) * 33],
                            lhsT=pB[:, t * 256 + 128:t * 256 + 256].bitcast(bf16),
                            rhs=Vh[:, t * 33:(t + 1) * 33],
                            start=True, stop=False)
                        # cur: slot t+1 keys, q_t = first half of slot t+1 block
                        nc.tensor.matmul(
                            otP[:, uu * 33:(uu + 1) * 33],
                            lhsT=pB[:, (t + 1) * 256:(t + 1) * 256 + 128].bitcast(bf16),
                            rhs=Vh[:, (t + 1) * 33:(t + 2) * 33],
                            start=False, stop=True)
                    nc.scalar.activation(
                        outW[:, tb * 264:(tb + 1) * 264], otP[:, :],
                        mybir.ActivationFunctionType.Copy)
                    nc.sync.dma_start(out=o[h][:, tb * 264:(tb + 1) * 264],
                                      in_=outW[:, tb * 264:(tb + 1) * 264])

        pending = None

        for g in range(4):
            Ks, Qs = [], []
            for ci in range(3):
                # the idle GpSimd engine's preamble ends earliest — its SWDGE
                # issues the critical first transfers ~2 µs sooner than HWDGE
                dma = nc.gpsimd if (g == 0 and ci == 0) else nc.sync
                ks0, ks1 = KSL[ci]
                Kc = qk_pool.tile([128, ks1 - ks0], bf16, tag=f"k{ci}",
                                  name=f"k{ci}g{g}")
                dma.dma_start(out=Kc[:, :], in_=kT[g][:, ks0:ks1])
                qs0, qs1 = QSL[ci]
                Qc = qk_pool.tile([128, qs1 - qs0], bf16, tag=f"q{ci}",
                                  name=f"q{ci}g{g}")
                dma.dma_start(out=Qc[:, :], in_=qT[g][:, qs0:qs1])
                Ks.append(Kc)
                Qs.append(Qc)
                if g == 0:
                    emit_bias(ci)       # need-order: K_ci, Q_ci, bias_ci

            def k_ap(ci, p0, s):
                lo = s * W - KSL[ci][0]
                return Ks[ci][p0:p0 + 32, lo:lo + W]

            def q_ap(ci, p0, w, nw):    # query windows w .. w+nw-1
                lo = w * W - QSL[ci][0]
                return Qs[ci][p0:p0 + 32, lo:lo + nw * W]

            for j in range(2):      # head pairs within group, strips (64j, 64j+32)
                pj = 2 * g + j      # pair index 0..7
                pair = [(4 * g + 2 * j + u, 32 * (2 * j + u)) for u in range(2)]
                on_pe = pj in PE_PAIRS
                # per-head fp32 sim in SBUF (DVE-evacuated pairs only)
                simS = {} if on_pe else {
                    h: sim_pool.tile([128, SIMW], f32, tag="simS",
                                     name=f"simS{h}")
                    for h, _ in pair}
                tS = {h: t_pool.tile([128, SIMW], f16, tag="tS", name=f"tS{h}")
                      for h, _ in pair}

                def emit_mm1(hs):
                    # mm1 (+ bias path) chunk by chunk
                    for ci, (cs, csz, slots) in enumerate(chunks):
                        lo2, hi2 = span(ci)
                        simPs = {h: sim_ps.tile([128, CHUNK], f32, tag="simP",
                                                name=f"simP{h}c{ci}")
                                 for h, _ in hs}
                        for s in slots:
                            off = s * 256 - cs
                            if s == 0:
                                # cols [0:128] (task -1) are filler: never
                                # computed, skipped by ACT, unread by mm2
                                rgn, qw_, qn = (128, 256), 0, 1
                            elif s == NSLOT - 1:
                                rgn, qw_, qn = (off, off + 128), s - 1, 1
                            else:
                                rgn, qw_, qn = (off, off + 256), s - 1, 2
                            for h, p0 in hs:
                                # on_pe: keep the psum group open so the bias
                                # matmul below accumulates (start=True re-marks
                                # the whole zero region pending, so the bias
                                # add must target exactly this slot's region)
                                nc.tensor.matmul(simPs[h][:, rgn[0]:rgn[1]],
                                                 lhsT=k_ap(ci, p0, s),
                                                 rhs=q_ap(ci, p0, qw_, qn),
                                                 start=True, stop=not on_pe,
                                                 tile_position=(p0, 0))
                            if on_pe:
                                for h, p0 in hs:
                                    nc.tensor.matmul(simPs[h][:, rgn[0]:rgn[1]],
                                                     lhsT=identT[:, :],
                                                     rhs=biasC[ci][:, rgn[0]:rgn[1]],
                                                     start=False, stop=True)
                        for h, _ in hs:
                            if on_pe:
                                nc.scalar.activation(
                                    tS[h][:, cs + lo2:cs + hi2],
                                    simPs[h][:, lo2:hi2],
                                    mybir.ActivationFunctionType.Tanh,
                                    scale=1.0 / SOFTCLAMP)
                            else:
                                nc.vector.tensor_add(
                                    simS[h][:, cs + lo2:cs + hi2],
                                    simPs[h][:, lo2:hi2],
                                    biasC[ci][:, lo2:hi2],
                                )

                if pj == 0:
                    # head-serial at the very start: h0's sim completes (and
                    # ACT starts) ~3 µs earlier
                    emit_mm1([pair[0]])
                    emit_mm1([pair[1]])
                else:
                    emit_mm1(pair)
                # previous pair's exp lands on the DVE queue right
                # after this pair's evacuation adds
                pI = {h: p_pool.tile([128, SIMW], i16, tag="pI", name=f"pI{h}")
                      for h, _ in pair}
                if pending is not None:
                    emit_schr(pending)
                # softcap: one ACT pass (tanh -> fp16), skipping the 128
                # filler cols at each end (DVE pairs; PE pairs ran it above)
                halves = ((128, 2176), (2176, SIMW - 128)) if pj == 7 \
                    else ((128, SIMW - 128),)
                for h, _ in pair:
                    if not on_pe:
                        for lo_, hi_ in halves:
                            nc.scalar.activation(
                                tS[h][:, lo_:hi_],
                                simS[h][:, lo_:hi_],
                                mybir.ActivationFunctionType.Tanh,
                                scale=1.0 / SOFTCLAMP)
                # V DMAs issued after the K/Q/bias stream (mm2 needs them
                # only ~15 µs later; keeps them off the critical early DMAs)
                Vhs = {}
                for h, p0 in pair:
                    Vh = v_pool.tile([128, NSLOT * 33], bf16, tag="vh",
                                     name=f"vh{h}")
                    nc.sync.dma_start(out=Vh[:, :], in_=vv[h])
                    Vhs[h] = Vh
                if pending is not None:
                    emit_mm2(pending)
                pending = {"pair": pair, "tS": tS, "pI": pI, "Vhs": Vhs}
        # drain the pipeline: last pair, head- and half-interleaved so the
        # tail chain (tanh -> exp -> mm2 -> copy -> DMA) pipelines
        if pending is not None:
            last = pending
            for u, (h, p0) in enumerate(last["pair"]):
                one = {"pair": [(h, p0)], "tS": last["tS"], "pI": last["pI"],
                       "Vhs": last["Vhs"]}
                for half, (lo_, hi_) in enumerate(((128, 2176),
                                                   (2176, SIMW - 128))):
                    nc.vector.tensor_scalar(
                        last["pI"][h][:, lo_:hi_],
                        last["tS"][h][:, lo_:hi_],
                        A_CONST, B_CONST,
                        mybir.AluOpType.mult, mybir.AluOpType.add)
                    emit_mm2(one, tbs=(half,))
    nc.compile()
    return nc


def _get_compiled():
    global _COMPILED
    if _COMPILED is None:
        _COMPILED = _build_bass()
    return _COMPILED


def _prep_core(c, qs, ks, vs, ab, mvec):
    """Build per-core input arrays. qs,ks,vs: (H, N, D) (qs pre-scaled)."""
    w0 = c * WPC
    qw = qs.reshape(H, NW, W, D)[:, w0:w0 + WPC]          # (H,16,128,32)
    qTc = np.ascontiguousarray(
        qw.reshape(4, 4, WPC, W, D).transpose(0, 1, 4, 2, 3).reshape(4, 128, WPC * W))

    kw = ks.reshape(H, NW, W, D)
    vw = vs.reshape(H, NW, W, D)
    khalo = np.zeros((H, NSLOT, W, D), np.float32)
    vhalo = np.zeros((H, NSLOT, W, D), np.float32)
    lo = w0 - 1
    src_lo = max(lo, 0)
    dst_lo = src_lo - lo
    khalo[:, dst_lo:] = kw[:, src_lo:w0 + WPC]
    vhalo[:, dst_lo:] = vw[:, src_lo:w0 + WPC]
    kTc = np.ascontiguousarray(
        khalo.reshape(4, 4, NSLOT, W, D).transpose(0, 1, 4, 2, 3).reshape(4, 128, NSLOT * W))
    vvc = np.concatenate([vhalo, np.ones((H, NSLOT, W, 1), np.float32)], axis=3)
    vvc = np.ascontiguousarray(
        vvc.transpose(0, 2, 1, 3).reshape(H, 128, NSLOT * 33))

    # bias, slot-major: slot s block cols = [cur-bias(task s-1) | prev-bias(task s)]
    # both halves use keys of global window w0+s-1; fold key mask (+ structural
    # masking of window -1) as additive penalty.
    bTc = np.zeros((128, NSLOT, 2, W), np.float32)         # (key, slot, half, q)
    for s in range(NSLOT):
        gw = w0 + s - 1
        if s > 0:
            bTc[:, s, 0, :] = ab[gw, :, 128:256].T          # cur role for task s-1
        if s < NSLOT - 1:
            bTc[:, s, 1, :] = ab[gw + 1, :, 0:128].T        # prev role for task s
        if gw < 0:
            pen = np.full((W,), MASK_PEN, np.float32)
        else:
            pen = np.where(mvec[gw * W:(gw + 1) * W], np.float32(0),
                           np.float32(MASK_PEN))
        bTc[:, s, :, :] += pen[:, None, None]
    bTc = np.ascontiguousarray(bTc.reshape(128, SIMW))
    return {"qT": qTc.astype(BF16), "kT": kTc.astype(BF16),
            "vv": vvc.astype(BF16), "bT": bTc.astype(BF16),
            "ident": np.eye(128, dtype=BF16)}


def _run_device(in_maps, trace=False):
    from concourse.bass_utils import run_bass_kernel_spmd
    nc = _get_compiled()
    res = run_bass_kernel_spmd(nc, in_maps, list(range(NCORES)), trace=trace)
    return res


def kernel(q, k, v, mask, attn_bias, memory_kv, _trace=False, _ret_res=False):
    q = np.asarray(q, np.float32)
    k = np.asarray(k, np.float32)
    v = np.asarray(v, np.float32)
    mask = np.asarray(mask)
    attn_bias = np.asarray(attn_bias, np.float32)
    memory_kv = np.asarray(memory_kv, np.float32)

    qs = q[0] * np.float32(SCALE)       # (H, N, D)
    ks, vs = k[0], v[0]
    ab = attn_bias[0]                   # (NW, W, 2W)
    mvec = mask[0].astype(bool)         # (N,)

    in_maps = [_prep_core(c, qs, ks, vs, ab, mvec) for c in range(NCORES)]
    res = _run_device(in_maps, trace=_trace)
    outs = [np.asarray(r["o"], dtype=np.float32) for r in res.results]

    big = np.stack(outs)                              # (8, H, 128, 528)
    # (core, h, q, task, 33) -> (h, core, task, q, 33) -> (h, n, 33)
    arr = big.reshape(NCORES, H, W, WPC, 33).transpose(1, 0, 3, 2, 4)
    arr = arr.reshape(H, N, 33)
    num = arr[..., :D].astype(np.float64)             # (H, N, D)
    z = arr[..., D].astype(np.float64)                # (H, N)

    # memory-slot attention (4 keys, no bias, mask=True) on host
    mk, mv = memory_kv[0], memory_kv[1]               # (H, 4, D)
    sim_m = np.einsum('hnd,hmd->hnm', qs, mk, dtype=np.float64)
    pm = np.exp(SOFTCLAMP * np.tanh(sim_m / SOFTCLAMP))
    num = num + np.einsum('hnm,hmd->hnd', pm, mv.astype(np.float64))
    z = z + pm.sum(-1)

    out = (num / z[..., None]).astype(np.float32)[None]   # (1, H, N, D)
    if _ret_res:
        return out, res
    return out
